# revision 25
# baseline (speedup 1.0000x reference)
"""Trainium2 Bass kernel for nn_LocalDenseCrossReadout (v5).

Strategy:
- Data-parallel over batch: 8 batches -> 8 NeuronCores, one batch per core.
- Host-side (numpy, O(D^2)): FiLM conditioning, folding of LayerNorm affine +
  FiLM + score scale into projection weights, v-bias folded into output bias,
  transposed multiplicative 0/1 mask blocks.
- Device, all bf16 matmul inputs (fp32 PSUM accumulate):
  * warmup matmuls on junk data keep the PE HAM clock-gate at K=8/8 while
    LayerNorm runs
  * LayerNorm stats+apply on natural-layout tiles (DVE/ACT split)
  * x^T via hardware DMA-transpose (no PE transposes of x)
  * projections with weights stationary (FWL), wide PSUM evacuations with
    per-partition bias on the activation engine
  * attention fully in transposed (kv-major) layout: S^T, gl^T, P^T, AV^T --
    no DMA transposes in the attention inner loop; softmax denominator via
    ones-matmul on the PE; normalization folded into the final evacuation
    scale; out bias added as a broadcast tensor_add.
"""

import sys

sys.path.insert(0, "/opt/trn_rl_repo")

import numpy as np
import ml_dtypes

import concourse.bass as bass
import concourse.tile as tile
from concourse import bacc
from concourse import mybir
from concourse.bass_utils import run_bass_kernel_spmd
from concourse.masks import make_identity

DIM, QS, QT, KS, KT, WIN, B, RANK = 512, 64, 16, 256, 16, 4, 8, 32
Q = QS * QT  # 1024
K = KS * KT  # 4096
WINW = 768  # aligned kv window per 128-row q tile
NWB = WINW // 128  # 6 kv blocks per window
NQT = Q // 128  # 8 q tiles
NKT = K // 128  # 32 kv tiles
F32 = mybir.dt.float32
BF16 = mybir.dt.bfloat16
FT = mybir.ActivationFunctionType
ALU = mybir.AluOpType

BFNP = ml_dtypes.bfloat16

# kv window start (aligned to 128) per q tile
WSTARTS = [0, 384, 896, 1408, 1920, 2432, 2944, 3328]
N_WARM = 56


def build_bass(debug=False, stage=4):
    nc = bacc.Bacc("TRN2", target_bir_lowering=False)
    q = nc.dram_tensor("q", [128, NQT, DIM], BF16, kind="ExternalInput")
    s = nc.dram_tensor("s", [128, NKT, DIM], BF16, kind="ExternalInput")
    wq = nc.dram_tensor("wq", [DIM, DIM], BF16, kind="ExternalInput")
    wk = nc.dram_tensor("wk", [DIM, DIM], BF16, kind="ExternalInput")
    wv = nc.dram_tensor("wv", [DIM, DIM], BF16, kind="ExternalInput")
    wo = nc.dram_tensor("wo", [DIM, DIM], BF16, kind="ExternalInput")
    wgq = nc.dram_tensor("wgq", [DIM, RANK], BF16, kind="ExternalInput")
    wgk = nc.dram_tensor("wgk", [DIM, RANK], BF16, kind="ExternalInput")
    rqt = nc.dram_tensor("rqt", [128, 4], F32, kind="ExternalInput")
    rkt = nc.dram_tensor("rkt", [128, 4], F32, kind="ExternalInput")
    bos = nc.dram_tensor("bos", [128, DIM], F32, kind="ExternalInput")
    m01 = nc.dram_tensor("m01", [NQT, 128, WINW], BF16, kind="ExternalInput")
    out = nc.dram_tensor("out", [Q, DIM], F32, kind="ExternalOutput")
    wrm = nc.dram_tensor("wrm", [128, 8], F32, kind="ExternalOutput")

    with tile.TileContext(nc) as tc:
        with (
            tc.tile_pool(name="consts", bufs=1) as consts,
            tc.tile_pool(name="wts", bufs=1) as wts,
            tc.tile_pool(name="big", bufs=1) as bigp,
            tc.tile_pool(name="xin", bufs=2) as xin,
            tc.tile_pool(name="xt", bufs=2) as xtp,
            tc.tile_pool(name="stats", bufs=2) as stats,
            tc.tile_pool(name="attn", bufs=2) as attn,
            tc.tile_pool(name="outp", bufs=2) as outp,
            tc.tile_pool(name="ps_big", bufs=2, space="PSUM") as ps_big,
            tc.tile_pool(name="ps_av", bufs=2, space="PSUM") as ps_av,
        ):
            # ---------------- constants + warmup ----------------
            eps = consts.tile([128, 1], F32)
            nc.vector.memset(eps, 1e-5)
            onec = consts.tile([128, 1], BF16)
            nc.vector.memset(onec, 1.0)
            ident = consts.tile([128, 128], F32)
            make_identity(nc, ident)
            warm_w = consts.tile([128, 128], BF16)
            nc.vector.memset(warm_w, 0.01)
            warm_x = consts.tile([128, 512], BF16)
            nc.vector.memset(warm_x, 0.01)
            last_warm = [None]

            def heartbeat(n):
                for _ in range(n):
                    wp = ps_av.tile([128, 640], F32, tag="av")
                    nc.tensor.matmul(wp[:, :512], warm_w, warm_x,
                                     start=True, stop=True)
                    last_warm[0] = wp

            heartbeat(N_WARM)

            rqt_sb = consts.tile([128, 4], F32)
            nc.gpsimd.dma_start(out=rqt_sb, in_=rqt[:, :])
            rkt_sb = consts.tile([128, 4], F32)
            nc.gpsimd.dma_start(out=rkt_sb, in_=rkt[:, :])
            bos_sb = consts.tile([128, DIM], F32)
            nc.gpsimd.dma_start(out=bos_sb, in_=bos[:, :])

            # weights as lhsT chunks: [128 (d_in within chunk c), c, d_out]
            def load_w(name, dram, n_out):
                t = wts.tile([128, 4, n_out], BF16, tag=name)
                nc.gpsimd.dma_start(
                    out=t, in_=dram.rearrange("(c p) d -> p c d", p=128))
                return t

            wq_sb = load_w("wq", wq, DIM)
            wk_sb = load_w("wk", wk, DIM)
            wv_sb = load_w("wv", wv, DIM)
            wo_sb = load_w("wo", wo, DIM)
            wgq_sb = load_w("wgq", wgq, RANK)
            wgk_sb = load_w("wgk", wgk, RANK)

            # persistent activation tensors
            qpT = bigp.tile([128, 4, Q], BF16, tag="qpT")     # q_p^T d-chunks
            kTp = bigp.tile([128, 4, K], BF16, tag="kTp")     # k_p^T d-chunks
            vb = bigp.tile([128, NKT, DIM], BF16, tag="vb")   # v rows (no bias)
            gqT = bigp.tile([32, Q], BF16, tag="gqT")
            gkT = bigp.tile([32, K], BF16, tag="gkT")
            qT = bigp.tile([128, 4 * NQT, 128], BF16, tag="qT")  # ln(q)^T chunks

            # ---- LayerNorm a group of ntile 128-row tiles in-place ----
            def ln_group(x_g, ntile):
                st6 = stats.tile([128, 8, 6], F32, tag="st6")
                mv = stats.tile([128, 8, 2], F32, tag="mv")
                for t in range(ntile):
                    nc.vector.bn_stats(out=st6[:, t, :], in_=x_g[:, t, :])
                    nc.vector.bn_aggr(out=mv[:, t, :], in_=st6[:, t, :])
                sd = stats.tile([128, 8], F32, tag="sd")
                nc.scalar.activation(out=sd[:, :ntile], in_=mv[:, :ntile, 1],
                                     func=FT.Sqrt, bias=eps, scale=1.0)
                rstd = stats.tile([128, 8], F32, tag="rstd")
                nc.vector.reciprocal(out=rstd[:, :ntile], in_=sd[:, :ntile])
                nmr = stats.tile([128, 8], F32, tag="nmr")
                nc.vector.scalar_tensor_tensor(
                    out=nmr[:, :ntile], in0=mv[:, :ntile, 0], scalar=-1.0,
                    in1=rstd[:, :ntile], op0=ALU.mult, op1=ALU.mult)
                for t in range(ntile):
                    if t % 2 == 0:
                        nc.vector.tensor_scalar(
                            out=x_g[:, t, :], in0=x_g[:, t, :],
                            scalar1=rstd[:, t:t + 1], scalar2=nmr[:, t:t + 1],
                            op0=ALU.mult, op1=ALU.add)
                    else:
                        nc.scalar.activation(
                            out=x_g[:, t, :], in_=x_g[:, t, :], func=FT.Identity,
                            scale=rstd[:, t:t + 1], bias=nmr[:, t:t + 1])

            # ---------------- q side: LN + transpose + projections ----------------
            xq = xin.tile([128, NQT, DIM], BF16, tag="xq", bufs=1)
            nc.sync.dma_start(out=xq, in_=q[:, :, :])
            ln_group(xq, NQT)
            for h in range(2):
                nc.sync.dma_start_transpose(
                    out=qT[:, 16 * h:16 * (h + 1), :], in_=xq[:, 4 * h:4 * (h + 1), :])
            qTr = qT.rearrange("p (t c) f -> p c t f", c=4)

            for m in range(4):
                pp = ps_big.tile([128, 1024], F32, tag="bigps")
                for g in range(2):
                    for c in range(4):
                        nc.tensor.matmul(pp[:, 512 * g:512 * (g + 1)],
                                         wq_sb[:, c, m * 128:(m + 1) * 128],
                                         qTr[:, c, 4 * g:4 * (g + 1), :],
                                         start=(c == 0), stop=(c == 3))
                nc.scalar.activation(out=qpT[:, m, :], in_=pp,
                                     func=FT.Identity, bias=rqt_sb[:, m:m + 1])
            for g in range(2):
                gbig = ps_big.tile([128, 1024], F32, tag="bigps")
                gp = gbig[0:32, 0:512]
                for c in range(4):
                    nc.tensor.matmul(gp, wgq_sb[:, c, :],
                                     qpT[:, c, 512 * g:512 * (g + 1)],
                                     start=(c == 0), stop=(c == 3))
                nc.vector.tensor_copy(gqT[:, 512 * g:512 * (g + 1)], gp)

            heartbeat(8)

            # ---------------- s side: 4 groups of 8 tiles ----------------
            for g in range(4):
                xs = xin.tile([128, 8, DIM], BF16, tag="xs")
                nc.sync.dma_start(out=xs, in_=s[:, 8 * g:8 * (g + 1), :])
                ln_group(xs, 8)
                sT = xtp.tile([128, 32, 128], BF16, tag="sT")
                nc.sync.dma_start_transpose(out=sT, in_=xs)
                sTr = sT.rearrange("p (t c) f -> p c t f", c=4)
                # k_p^T for these 1024 kv cols
                for m in range(4):
                    pp = ps_big.tile([128, 1024], F32, tag="bigps")
                    for h in range(2):
                        for c in range(4):
                            nc.tensor.matmul(pp[:, 512 * h:512 * (h + 1)],
                                             wk_sb[:, c, m * 128:(m + 1) * 128],
                                             sTr[:, c, 4 * h:4 * (h + 1), :],
                                             start=(c == 0), stop=(c == 3))
                    nc.scalar.activation(out=kTp[:, m, 1024 * g:1024 * (g + 1)],
                                         in_=pp, func=FT.Identity,
                                         bias=rkt_sb[:, m:m + 1])
                # v rows (no bias; folded into bos), two tiles per psum
                for u in range(4):
                    pv = ps_big.tile([128, 1024], F32, tag="bigps")
                    for h in range(2):
                        for c in range(4):
                            nc.tensor.matmul(pv[:, 512 * h:512 * (h + 1)],
                                             sTr[:, c, 2 * u + h, :], wv_sb[:, c, :],
                                             start=(c == 0), stop=(c == 3))
                    nc.scalar.copy(vb[:, 8 * g + 2 * u:8 * g + 2 * u + 2, :],
                                   pv.rearrange("p (a b) -> p a b", a=2))
                # gate_k^T
                for h in range(2):
                    gbig = ps_big.tile([128, 1024], F32, tag="bigps")
                    gp = gbig[0:32, 0:512]
                    for c in range(4):
                        nc.tensor.matmul(
                            gp, wgk_sb[:, c, :],
                            kTp[:, c, 1024 * g + 512 * h:1024 * g + 512 * (h + 1)],
                            start=(c == 0), stop=(c == 3))
                    nc.vector.tensor_copy(
                        gkT[:, 1024 * g + 512 * h:1024 * g + 512 * (h + 1)], gp)
                heartbeat(6)

            # ------------- attention, transposed layout, lookahead pipeline ------
            msks = []
            for t in range(NQT):
                msk = attn.tile([128, WINW], BF16, tag="msk", bufs=4)
                nc.gpsimd.dma_start(out=msk, in_=m01[t, :, :])
                msks.append(msk)

            LOOK = 2
            state = {}

            def attn_front(t):
                w0 = WSTARTS[t]
                qc = bass.ts(t, 128)
                kb0 = w0 // 128
                # gate logits^T [kv, q]: 6 blocks, contraction 32
                gl = ps_big.tile([128, 1024], F32, tag="bigps")
                for j in range(NWB):
                    nc.tensor.matmul(gl[:, 128 * j:128 * (j + 1)],
                                     gkT[:, w0 + 128 * j:w0 + 128 * (j + 1)],
                                     gqT[:, qc], start=True, stop=True)
                th = attn.tile([128, WINW], BF16, tag="th", bufs=3)
                nc.scalar.activation(out=th, in_=gl[:, :WINW], func=FT.Tanh, scale=0.5)
                gm = attn.tile([128, WINW], BF16, tag="gm", bufs=3)
                nc.vector.scalar_tensor_tensor(out=gm, in0=th, scalar=1.0,
                                               in1=msks[t], op0=ALU.add, op1=ALU.mult)
                # scores^T [kv, q]
                sc = ps_big.tile([128, 1024], F32, tag="bigps")
                for j in range(NWB):
                    for c in range(4):
                        nc.tensor.matmul(sc[:, 128 * j:128 * (j + 1)],
                                         kTp[:, c, w0 + 128 * j:w0 + 128 * (j + 1)],
                                         qpT[:, c, qc], start=(c == 0), stop=(c == 3))
                ex = attn.tile([128, WINW], BF16, tag="ex", bufs=3)
                nc.scalar.activation(out=ex, in_=sc[:, :WINW], func=FT.Exp)
                P = attn.tile([128, WINW], BF16, tag="P", bufs=3)
                nc.vector.tensor_mul(P, ex, gm)
                Pb = P.rearrange("p (j f) -> p j f", j=NWB)
                # attn^T @ V blocks + softmax denominator row
                av = ps_av.tile([128, 640], F32, tag="av")
                for db in range(4):
                    for j in range(NWB):
                        nc.tensor.matmul(
                            av[:, 128 * db:128 * (db + 1)],
                            vb[:, kb0 + j, 128 * db:128 * (db + 1)],
                            Pb[:, j, :], start=(j == 0), stop=(j == NWB - 1))
                for j in range(NWB):
                    nc.tensor.matmul(av[0:1, 512:640], onec, Pb[:, j, :],
                                     start=(j == 0), stop=(j == NWB - 1))
                avT = attn.tile([128, 4, 128], BF16, tag="avT", bufs=3)
                nc.scalar.copy(avT, av[:, :512].rearrange("p (a b) -> p a b", a=4))
                rsr = stats.tile([1, 128], F32, tag="rsr", bufs=4)
                nc.vector.reciprocal(out=rsr, in_=av[0:1, 512:640])
                state[t] = (avT, rsr)

            def attn_back(t):
                avT, rsr = state.pop(t)
                # out = (avT^T @ Wo) * rinv + bos
                fin = ps_av.tile([128, 640], F32, tag="av")
                for c in range(4):
                    nc.tensor.matmul(fin[:, :512], avT[:, c, :], wo_sb[:, c, :],
                                     start=(c == 0), stop=(c == 3))
                nc.tensor.transpose(fin[:, 512:513], rsr, ident[0:1, 0:1])
                rinv = stats.tile([128, 1], F32, tag="rinv", bufs=4)
                nc.scalar.copy(rinv, fin[:, 512:513])
                ob = outp.tile([128, DIM], F32, tag="ob")
                nc.vector.scalar_tensor_tensor(out=ob, in0=fin[:, :512], scalar=rinv,
                                               in1=bos_sb, op0=ALU.mult, op1=ALU.add)
                nc.gpsimd.dma_start(out=out[t * 128:(t + 1) * 128, :], in_=ob)

            for t in range(NQT + LOOK):
                if t < NQT:
                    attn_front(t)
                if t >= LOOK:
                    attn_back(t - LOOK)

            wsb = consts.tile([128, 8], F32)
            nc.scalar.copy(wsb, last_warm[0][:, :8])
            nc.gpsimd.dma_start(out=wrm[:, :], in_=wsb)

    if not nc.is_finalized():
        nc.finalize()
    return nc


_NC_CACHE = {}


def _get_nc(debug=False, stage=4):
    key = (debug, stage)
    if key not in _NC_CACHE:
        _NC_CACHE[key] = build_bass(debug=debug, stage=stage)
    return _NC_CACHE[key]


def _host_fold(inputs):
    f32 = np.float32
    scale = f32(DIM ** -0.5)
    ctx0 = np.asarray(inputs["ctx0"], f32)
    ctx1 = np.asarray(inputs["ctx1"], f32)
    pre = ctx0 @ inputs["Wc0"] + inputs["bc0"] + ctx1 @ inputs["Wc1"] + inputs["bc1"]
    pre = np.asarray(pre, f32)
    h = pre / (1.0 + np.exp(-pre))
    gb = np.asarray(h @ inputs["Wf"] + inputs["bf"], f32)
    gamma, beta = gb[:, :DIM], gb[:, DIM:]

    qn_g = np.asarray(inputs["qn_g"], f32)
    qn_b = np.asarray(inputs["qn_b"], f32)
    kvn_g = np.asarray(inputs["kvn_g"], f32)
    kvn_b = np.asarray(inputs["kvn_b"], f32)
    Wq, bq = np.asarray(inputs["Wq"], f32), np.asarray(inputs["bq"], f32)
    Wk, bk = np.asarray(inputs["Wk"], f32), np.asarray(inputs["bk"], f32)
    Wv, bv = np.asarray(inputs["Wv"], f32), np.asarray(inputs["bv"], f32)
    Wo, bo = np.asarray(inputs["Wo"], f32), np.asarray(inputs["bo"], f32)
    mask = np.asarray(inputs["mask"], f32)

    WkS = np.ascontiguousarray((Wk * kvn_g[:, None]).astype(BFNP))
    r_k = (kvn_b @ Wk + bk).astype(f32)
    WvS = np.ascontiguousarray((Wv * kvn_g[:, None]).astype(BFNP))
    r_v = (kvn_b @ Wv + bv).astype(f32)
    WgqS = np.ascontiguousarray((inputs["Wgq"] / scale / np.sqrt(RANK)).astype(BFNP))
    WgkS = np.ascontiguousarray(np.asarray(inputs["Wgk"], BFNP))
    WoS = np.ascontiguousarray(Wo.astype(BFNP))
    bosv = np.ascontiguousarray(
        np.broadcast_to((r_v @ Wo + bo).astype(f32), (128, DIM)))

    # transposed multiplicative 0/1 mask (x0.5 folds the tanh->sigmoid affine)
    # m01[t][p, 128*j + f] = 0.5 * (mask[128*t + f, w0 + 128*j + p] == 0)
    m01 = np.empty((NQT, 128, WINW), dtype=BFNP)
    for t, w in enumerate(WSTARTS):
        blk = (mask[t * 128:(t + 1) * 128, w:w + WINW] == 0.0) * 0.5  # [q, kv]
        blkT = blk.T.reshape(NWB, 128, 128).transpose(1, 0, 2)  # [kvp, j, q]
        m01[t] = blkT.reshape(128, WINW).astype(BFNP)

    query = np.asarray(inputs["query"], f32).reshape(B, Q, DIM)
    source = np.asarray(inputs["source"], f32).reshape(B, K, DIM)
    # device x layout: [128 partitions, tile, 512]
    qdev = np.ascontiguousarray(
        query.reshape(B, NQT, 128, DIM).transpose(0, 2, 1, 3).astype(BFNP))
    sdev = np.ascontiguousarray(
        source.reshape(B, NKT, 128, DIM).transpose(0, 2, 1, 3).astype(BFNP))

    in_maps = []
    for b in range(B):
        sg = (qn_g * (1.0 + gamma[b])).astype(f32)
        WqS = np.ascontiguousarray((Wq * sg[:, None] * scale).astype(BFNP))
        r_q = (((qn_b * (1.0 + gamma[b]) + beta[b]) @ Wq + bq) * scale).astype(f32)
        in_maps.append({
            "q": qdev[b],
            "s": sdev[b],
            "wq": WqS, "wk": WkS, "wv": WvS, "wo": WoS,
            "wgq": WgqS, "wgk": WgkS,
            "rqt": np.ascontiguousarray(r_q.reshape(4, 128).T),
            "rkt": np.ascontiguousarray(r_k.reshape(4, 128).T),
            "bos": bosv,
            "m01": m01,
        })
    return in_maps


def kernel(**inputs):
    nc = _get_nc()
    in_maps = _host_fold(inputs)
    res = run_bass_kernel_spmd(nc, in_maps, core_ids=list(range(B)))
    out = np.stack([res.results[b]["out"] for b in range(B)])
    return out.reshape(B, QS, QT, DIM).astype(np.float32)


if __name__ == "__main__":
    build_bass()
    print("bass build OK")
